# revision 54
# baseline (speedup 1.0000x reference)
"""Trainium2 Bass kernel: causal attention with 3D (Rodrigues) RoPE.

Sharding: tensor-parallel over heads (2 heads/core on 8 cores) for
QKV projection + RoPE + SDPA, then an AllToAll redistributes attention
outputs so the output projection is sharded over tokens (512/core).
The A2A is split per local head so the first half overlaps attention,
and the output projection's contraction is split to match.

All matmul operands are bf16 (fp32 PSUM accumulation).

Structure: chunks 0-3 (batch 0) project alone; chunks 4-7 (batch 1)
interleave with batch-0 attention groups so exp/ACT hides under
projection matmuls and the PE never idles across the phase boundary.
Batch-1 attention runs afterwards with paired off-diagonal exps, then
the split output projection overlaps the second AllToAll.

Layouts (per core):
  x^T       [1536, 4096]   tokens on the free axis
  q^T,k^T   3 aligned M-tiles (384 rows = q0 q1 k0 k1, plane-major
            triplet order so RoPE shifts are 32-row blocks)
  V         [4096, 194]    computed DIRECTLY token-major (x-tile
            stationary, w_v columns moving); col 96/193 hold ones so
            the softmax denominator is row 96 of the PV matmul
  S^T       [tk=128, tq=512] softmax's reduction axis = PE contraction
            axis -> no transposes anywhere.
"""

import sys

sys.path.insert(0, "/opt/trn_rl_repo")

import numpy as np

D_MODEL, N_HEADS, HEAD_DIM, MAX_POS = 1536, 16, 96, 4096
B, T = 2, 2048
NTOK = B * T                      # 4096
NCORES = 8
HPC = N_HEADS // NCORES           # 2 heads per core
NTRIP = HEAD_DIM // 3             # 32 triplets
KT = D_MODEL // 128               # 12 contraction tiles
NCH = NTOK // 512                 # 8 token chunks of 512
TQC = T // 512                    # 4 query chunks per batch
SCALE = 1.0 / np.sqrt(HEAD_DIM)

_CACHE = {}


def _build_nc():
    import concourse.bass as bass
    import concourse.mybir as mybir
    import concourse.tile as tile
    from concourse import bacc

    f32 = mybir.dt.float32
    bf16 = mybir.dt.bfloat16
    MUL = mybir.AluOpType.mult
    ADD = mybir.AluOpType.add
    CP = mybir.ActivationFunctionType.Copy
    EXP = mybir.ActivationFunctionType.Exp

    nc = bacc.Bacc("TRN2", target_bir_lowering=False, debug=False,
                   enable_asserts=False, num_devices=NCORES)

    xT = nc.dram_tensor("xT", [D_MODEL, NTOK], bf16, kind="ExternalInput").ap()
    wallT = nc.dram_tensor("wallT", [D_MODEL, 576], bf16, kind="ExternalInput").ap()
    woT = nc.dram_tensor("woT", [D_MODEL, D_MODEL], bf16, kind="ExternalInput").ap()
    cco = nc.dram_tensor("cco", [96, 3, T], bf16, kind="ExternalInput").ap()
    msk = nc.dram_tensor("msk", [128, 128], bf16, kind="ExternalInput").ap()
    out = nc.dram_tensor("out", [D_MODEL, 512], bf16, kind="ExternalOutput").ap()

    with tile.TileContext(nc) as tc:
        with tc.tile_pool(name="dram", bufs=1, space="DRAM") as dram:
            a2a_in = [dram.tile([NCH, 96, 512], bf16, name=f"a2a_in{h}")
                      for h in range(HPC)]
            a2a_out = [dram.tile([NCH, 96, 512], bf16, name=f"a2a_out{h}")
                       for h in range(HPC)]

            with tc.tile_pool(name="pp", bufs=1) as pp, \
                 tc.tile_pool(name="p1s", bufs=2) as p1s, \
                 tc.tile_pool(name="p2", bufs=5) as p2, \
                 tc.tile_pool(name="p2b", bufs=3) as p2b, \
                 tc.tile_pool(name="p2c", bufs=1) as p2c:

                # [96, chunk, tensor(q0 q1 k0 k1), col]
                qkr = pp.tile([96, NCH, 4, 512], bf16, tag="qkr")
                v_sb = pp.tile([128, NTOK // 128, 194], bf16, tag="vsb")
                wall_sb = pp.tile([128, KT, 576], bf16, tag="wall")
                m_sb = pp.tile([128, 128], bf16, tag="msb")
                partA_sb = p2c.tile([128, KT, 512], bf16, tag="partA")

                # ---------------- shared emitters ----------------
                def load_pair(cp):
                    """Load chunks 2cp, 2cp+1 as one [128,KT,1024] tile:
                    2KB partition lines halve the DMA descriptor count."""
                    coff = ((2 * cp) % TQC) * 512
                    xt = p1s.tile([128, KT, 1024], bf16, tag="xt",
                                  name="xt")
                    engs = [nc.sync, nc.gpsimd, nc.scalar,
                            nc.sync, nc.gpsimd, nc.scalar]
                    if cp == 0:
                        # first pair: chunk 0's columns land first so the
                        # projection can start before chunk 1 arrives
                        for half in range(2):
                            for kp in range(KT // 2):
                                engs[kp].dma_start(
                                    xt[:, 2 * kp:2 * kp + 2,
                                       half * 512:(half + 1) * 512],
                                    xT[kp * 256:(kp + 1) * 256,
                                       half * 512:(half + 1) * 512]
                                    .rearrange("(a b) c -> b a c", a=2))
                    else:
                        for kp in range(KT // 2):
                            engs[kp].dma_start(
                                xt[:, 2 * kp:2 * kp + 2, :],
                                xT[kp * 256:(kp + 1) * 256,
                                   cp * 1024:(cp + 1) * 1024]
                                .rearrange("(a b) c -> b a c", a=2))
                    c_sl = p1s.tile([96, 3, 1024], bf16, tag="csl",
                                    name="c_sl")
                    nc.scalar.dma_start(c_sl[:],
                                        cco[:, :, coff:coff + 1024])
                    return c_sl, xt

                def proj_chunk(ch, cur, qk_ps, v_ps, off=0):
                    """Generator: qk into qk_ps ([128,3,512] view), v into
                    v_ps ([128,2,192] per half); yields between MM bursts
                    so attention tiles can interleave."""
                    c_sl, xt = cur
                    for m in range(3):
                        for kt in range(KT):
                            nc.tensor.matmul(
                                qk_ps[:, m, :],
                                wall_sb[:, kt, m * 128:(m + 1) * 128],
                                xt[:, kt, off:off + 512],
                                start=(kt == 0), stop=(kt == KT - 1))
                            if kt % 6 == 5:
                                yield
                    for half in range(2):
                        vps = v_ps(half)
                        for gg in range(2):
                            g = half * 2 + gg
                            for kt in range(KT):
                                nc.tensor.matmul(
                                    vps[:, gg, :],
                                    xt[:, kt,
                                       off + g * 128:off + (g + 1) * 128],
                                    wall_sb[:, kt, 384:576],
                                    start=(kt == 0), stop=(kt == KT - 1),
                                    skip_group_check=True)
                                if kt % 6 == 5:
                                    yield
                        gb = ch * 4 + half * 2
                        nc.scalar.activation(
                            v_sb[:, gb:gb + 2, 0:96], vps[:, :, 0:96], CP)
                        nc.scalar.activation(
                            v_sb[:, gb:gb + 2, 97:193], vps[:, :, 96:192],
                            CP)

                    # straddled eviction of q0 q1 k0 k1
                    raws = p1s.tile([96, 4, 512], bf16, tag="raws",
                                    name="raws")
                    ev_act = [
                        (raws[0:96, 0, :], qk_ps[0:96, 0, :]),
                        (raws[0:32, 1, :], qk_ps[96:128, 0, :]),
                        (raws[32:64, 1, :], qk_ps[0:32, 1, :]),
                        (raws[64:96, 1, :], qk_ps[32:64, 1, :]),
                        (raws[0:64, 2, :], qk_ps[64:128, 1, :]),
                        (raws[64:96, 2, :], qk_ps[0:32, 2, :]),
                        (raws[64:96, 3, :], qk_ps[96:128, 2, :]),
                    ]
                    for dst_ap, src_ap in ev_act:
                        nc.scalar.activation(dst_ap, src_ap, CP)
                    nc.vector.tensor_copy(raws[0:32, 3, :],
                                          qk_ps[32:64, 2, :])
                    nc.vector.tensor_copy(raws[32:64, 3, :],
                                          qk_ps[64:96, 2, :])

                    # rope on all 4 tensors at once
                    dst = qkr[:, ch, :, :]
                    g1 = p1s.tile([96, 4, 512], bf16, tag="g1",
                                  bufs=1)
                    nc.gpsimd.dma_start(g1[0:64, :, :], raws[32:96, :, :])
                    nc.gpsimd.dma_start(g1[64:96, :, :], raws[0:32, :, :])
                    g2 = p1s.tile([96, 4, 512], bf16, tag="g2",
                                  bufs=1)
                    nc.gpsimd.dma_start(g2[0:32, :, :], raws[64:96, :, :])
                    nc.gpsimd.dma_start(g2[32:96, :, :], raws[0:64, :, :])

                    def cb(d):
                        return c_sl[:, d, off:off + 512].unsqueeze(
                            1).broadcast_to([96, 4, 512])

                    nc.vector.tensor_tensor(dst, raws[:], cb(0), MUL)
                    nc.vector.tensor_tensor(g1[:], g1[:], cb(1), MUL)
                    nc.vector.tensor_tensor(dst, dst, g1[:], ADD)
                    nc.vector.tensor_tensor(g2[:], g2[:], cb(2), MUL)
                    nc.vector.tensor_tensor(dst, dst, g2[:], ADD)

                # ---------------- attention emitters ----------------
                gp_state = {"after": None}

                def emit_pv(h, pend):
                    ti, tt, lo, pt_ap, pv, ntk, b, cl = pend.pop(0)
                    nc.tensor.matmul(
                        pv[0:97, lo:512],
                        v_sb[:, b * 16 + tt, h * 97:h * 97 + 97],
                        pt_ap[:, lo:512], start=(ti == 0),
                        stop=(ti == ntk - 1), skip_group_check=True)
                    if ti == ntk - 1:
                        return normalize(h, pv, b, cl)
                    return None

                def normalize(h, pv, b, cl):
                    lcp = p2b.tile([1, 512], f32, tag="lcp", name="lcp")
                    nc.vector.tensor_copy(lcp[:], pv[96:97, :])
                    linv = p2b.tile([1, 512], f32, tag="linv", name="linv")
                    nc.vector.reciprocal_approx_fast(linv[:], lcp[:])
                    brow = p2b.tile([96, 512], f32, tag="brow", name="brow")
                    # broadcast 1/denom across partitions via a sync-queue
                    # DMA (free-axis step-0 read) -> no gpsimd op, so the
                    # collective trigger cannot stall normalization
                    nc.sync.dma_start(
                        brow[:],
                        linv[:].unsqueeze(1).broadcast_to([1, 96, 512]))
                    att = p2b.tile([96, 512], bf16, tag="att", name="att")
                    nc.vector.tensor_tensor(att[:], pv[0:96, :], brow[:],
                                            MUL)
                    return nc.sync.dma_start(
                        a2a_in[h][b * TQC + cl, :, :], att[:])

                def attn_group(h, b, cl, pend, ps_s2, ps_pv):
                    """Generator: QK + exp + (lagged) PV for one (head,
                    batch, q-chunk); yields between tiles. ps_s2 None ->
                    unpaired off-diag tiles."""
                    qch = b * TQC + cl
                    pv = ps_pv.tile([128, 512], f32, tag="pv", name="pv")
                    ntk = 4 * cl + 4

                    def push(entry):
                        pend.append(entry)
                        if len(pend) > 3:
                            emit_pv(h, pend)

                    for dp in range(2):
                        sp2 = ps_s2.tile([128, 2, 512], f32, tag="s2",
                                         name="spd")
                        pt2 = p2.tile([128, 2, 512], bf16, tag="p2",
                                      name="ptd")
                        for j in range(2):
                            d = dp * 2 + j
                            tt = 4 * cl + d
                            kc = (tt % 4) * 128
                            lo = d * 128
                            nc.tensor.matmul(
                                sp2[:, j, lo:512],
                                qkr[:, qch, 2 + h, kc:kc + 128],
                                qkr[:, qch, h, lo:512],
                                start=True, stop=True)
                            nc.scalar.activation(pt2[:, j, lo:512],
                                                 sp2[:, j, lo:512], EXP)
                            nc.vector.tensor_tensor(
                                pt2[:, j, lo:lo + 128],
                                pt2[:, j, lo:lo + 128], m_sb[:], MUL)
                            push((d, tt, lo, pt2[:, j, :], pv, ntk, b, cl))
                        yield
                    if True:
                        for pr in range(2 * cl):
                            sp2 = ps_s2.tile([128, 2, 512], f32, tag="s2",
                                             name="sp2")
                            pt2 = p2.tile([128, 2, 512], bf16, tag="p2",
                                          name="pt2")
                            for j in range(2):
                                tt = 2 * pr + j
                                kch = b * TQC + tt // 4
                                kc = (tt % 4) * 128
                                nc.tensor.matmul(
                                    sp2[:, j, :],
                                    qkr[:, kch, 2 + h, kc:kc + 128],
                                    qkr[:, qch, h, 0:512],
                                    start=True, stop=True)
                            nc.scalar.activation(pt2[:], sp2[:], EXP)
                            for j in range(2):
                                tt = 2 * pr + j
                                push((4 + tt, tt, 0, pt2[:, j, :], pv,
                                      ntk, b, cl))
                            yield

                def drain(h, pend):
                    last_w = None
                    while pend:
                        w = emit_pv(h, pend)
                        last_w = w or last_w
                    return last_w

                def load_att2(half, after=None):
                    flat = a2a_out[half][:].rearrange("a b c -> (a b) c")
                    att2 = []
                    for ep in range(3):
                        t = p2c.tile([128, 2, 512], bf16,
                                     tag=f"att2_{half}_{ep}",
                                     name=f"att2_{half}_{ep}")
                        ld = nc.scalar.dma_start(
                            t[:], flat[ep * 256:(ep + 1) * 256, :]
                            .rearrange("(a b) c -> b a c", a=2))
                        if after is not None:
                            tile.add_dep_helper(
                                ld.ins, after.ins, sync=False,
                                reason="sync queue order")
                        att2.append(t[:, 0, :])
                        att2.append(t[:, 1, :])
                    return att2

                def oproj_group(half, g4, att2, ps_o):
                    pos = [ps_o.tile([128, 512], f32, tag="o",
                                     name=f"po_{half}_{g4}_{i}")
                           for i in range(4)]
                    for ep in range(3):
                        wot = p2c.tile([128, 2, 512], bf16, tag="wos",
                                       bufs=4, name="wot")
                        nc.sync.dma_start(
                            wot[:],
                            woT[half * 768 + ep * 256:
                                half * 768 + (ep + 1) * 256,
                                g4 * 512:(g4 + 1) * 512]
                            .rearrange("(a b) c -> b a c", a=2))
                        for j in range(2):
                            et = ep * 2 + j
                            for i in range(4):
                                nc.tensor.matmul(
                                    pos[i][:],
                                    wot[:, j, i * 128:(i + 1) * 128],
                                    att2[et][:], start=(et == 0),
                                    stop=(et == 5), skip_group_check=True)
                    for i in range(4):
                        dt_ = g4 * 4 + i
                        if half == 0:
                            nc.vector.tensor_copy(
                                partA_sb[:, dt_, :], pos[i][:])
                        else:
                            ot = p2b.tile([128, 512], bf16, tag="ot",
                                          bufs=3, name="ot")
                            nc.vector.tensor_tensor(
                                ot[:], pos[i][:], partA_sb[:, dt_, :], ADD)
                            nc.sync.dma_start(
                                out[dt_ * 128:(dt_ + 1) * 128, :], ot[:])

                # ---------------- schedule ----------------
                nc.sync.dma_start(m_sb[:], msk[:])
                for kq in range(KT // 4):
                    eng = nc.sync if kq % 2 == 0 else nc.scalar
                    eng.dma_start(wall_sb[:, 4 * kq:4 * kq + 4, :],
                                  wallT[kq * 512:(kq + 1) * 512, :]
                                  .rearrange("(a b) c -> b a c", a=4))
                nc.vector.memset(v_sb[:, :, 96:97], 1.0)
                nc.vector.memset(v_sb[:, :, 193:194], 1.0)

                pend = {0: [], 1: []}

                # --- section 1: chunks 0-3 (batch 0) project alone ---
                with tc.tile_pool(name="ps_qk", bufs=2,
                                  space="PSUM") as ps_qk, \
                     tc.tile_pool(name="ps_v", bufs=2, space="PSUM") as ps_v:
                    # PE warmup while first DMAs fly
                    ws = pp.tile([128, 256], bf16, tag="warm")
                    nc.vector.memset(ws[:], 0.0)
                    psw = ps_qk.tile([128, 3, 512], f32, tag="qk",
                                     name="psw")
                    for _ in range(72):
                        nc.tensor.matmul(psw[:, 0, 0:256], ws[:, 0:128],
                                         ws[:], start=True, stop=True)

                    cur = load_pair(0)
                    for ch in range(NCH):
                        if ch % 2 == 0 and ch + 2 < NCH:
                            nxt = load_pair((ch + 2) // 2)
                        qk_ps = ps_qk.tile([128, 3, 512], f32, tag="qk",
                                           name="ps")

                        def v_ps(half):
                            return ps_v.tile([128, 2, 192], f32,
                                             tag="vps", name="vps")

                        for _ in proj_chunk(ch, cur, qk_ps, v_ps,
                                            off=(ch % 2) * 512):
                            pass
                        if ch % 2 == 1 and ch + 1 < NCH:
                            cur = nxt

                # --- attention + A2A + o-proj ---
                with tc.tile_pool(name="ps_s2", bufs=3,
                                  space="PSUM") as ps_s2, \
                     tc.tile_pool(name="ps_pv", bufs=2,
                                  space="PSUM") as ps_pv:
                    order0 = [(b, cl) for cl in range(TQC - 1, -1, -1)
                              for b in range(B)]
                    for b, cl in order0:
                        for _ in attn_group(0, b, cl, pend[0],
                                            ps_s2, ps_pv):
                            pass
                    drain(0, pend[0])
                    with tc.high_priority():
                        trig1 = nc.gpsimd.collective_compute(
                            "AllToAll", mybir.AluOpType.bypass,
                            replica_groups=[list(range(NCORES))],
                            ins=[a2a_in[0].opt()], outs=[a2a_out[0].opt()])
                    gp_state["after"] = trig1
                    order1 = [(b, cl) for cl in range(TQC - 1, -1, -1)
                              for b in range(B)]
                    for b, cl in order1:
                        for _ in attn_group(1, b, cl, pend[1],
                                            ps_s2, ps_pv):
                            pass
                    last_w = drain(1, pend[1])
                att2A = load_att2(0, last_w)
                # A2A#2 runs on the collective engine while PE does oproj0
                with tc.high_priority():
                    nc.gpsimd.collective_compute(
                        "AllToAll", mybir.AluOpType.bypass,
                        replica_groups=[list(range(NCORES))],
                        ins=[a2a_in[1].opt()], outs=[a2a_out[1].opt()])
                with tc.tile_pool(name="ps_o", bufs=5,
                                  space="PSUM") as ps_o:
                    for g4 in range(3):
                        oproj_group(0, g4, att2A, ps_o)
                    att2B = load_att2(1)
                    for g4 in range(3):
                        oproj_group(1, g4, att2B, ps_o)

    nc.compile()
    return nc


def _plane_major(w):
    """Reorder head-dim rows 3k+i -> 32i+k (per 96-row head block)."""
    idx = np.empty(96, dtype=np.int64)
    for i in range(3):
        for k in range(NTRIP):
            idx[32 * i + k] = 3 * k + i
    return w[idx]


def _prep_inputs(x, w_qkv, w_o, Rs):
    x = np.asarray(x, dtype=np.float32)
    w_qkv = np.asarray(w_qkv, dtype=np.float32)
    w_o = np.asarray(w_o, dtype=np.float32)
    Rs = np.asarray(Rs, dtype=np.float32)

    import ml_dtypes
    bf = ml_dtypes.bfloat16

    xT = np.ascontiguousarray(x.reshape(NTOK, D_MODEL).T).astype(bf)

    # rope coefficients, plane-major rows: C[d, delta, t]
    R = Rs[:T]                                   # (T, 32, 3, 3)
    cco = np.empty((96, 3, T), dtype=np.float32)
    for d in range(3):
        for i in range(3):
            cco[32 * i:32 * i + 32, d, :] = R[:, :, i, (i + d) % 3].T
    cco = cco.astype(bf)

    # lower-triangular mask for the mixed 128x128 diagonal block
    j = np.arange(128)[:, None]
    i = np.arange(128)[None, :]
    msk = (j <= i).astype(bf)

    # w_o columns-for-even-heads first, then odd (matches split A2A halves)
    woT = np.ascontiguousarray(w_o.T)            # rows e = h*96+d
    perm = np.concatenate(
        [np.arange(h * 96, (h + 1) * 96) for h in range(0, 16, 2)] +
        [np.arange(h * 96, (h + 1) * 96) for h in range(1, 16, 2)])
    woTp = np.ascontiguousarray(woT[perm]).astype(bf)

    def w_row(s, h):                             # rows of w_qkv for (q/k/v, head)
        base = (s * N_HEADS + h) * HEAD_DIM
        return w_qkv[base:base + HEAD_DIM]

    in_maps = []
    for c in range(NCORES):
        h0, h1 = 2 * c, 2 * c + 1
        wall = np.concatenate([
            _plane_major(w_row(0, h0)) * SCALE,
            _plane_major(w_row(0, h1)) * SCALE,
            _plane_major(w_row(1, h0)),
            _plane_major(w_row(1, h1)),
            w_row(2, h0),
            w_row(2, h1),
        ], axis=0)                               # [576, 1536]
        wallT = np.ascontiguousarray(wall.T).astype(bf)  # [1536, 576]
        in_maps.append({
            "xT": xT, "wallT": wallT, "woT": woTp,
            "cco": cco, "msk": msk,
        })
    return in_maps


def kernel(x, w_qkv, w_o, Rs):
    from concourse import bass_utils

    if "nc" not in _CACHE:
        _CACHE["nc"] = _build_nc()
    nc = _CACHE["nc"]
    in_maps = _prep_inputs(x, w_qkv, w_o, Rs)
    res = bass_utils.run_bass_kernel_spmd(
        nc, in_maps, core_ids=list(range(NCORES)))
    full_T = np.concatenate(
        [np.asarray(res.results[c]["out"], dtype=np.float32)
         for c in range(NCORES)], axis=1)        # [1536, 4096]
    return np.ascontiguousarray(full_T.T).reshape(B, T, D_MODEL)
